# revision 2
# baseline (speedup 1.0000x reference)
"""LIF spiking network forward (nn_LIFSG) on 8 Trainium2 NeuronCores — v5.

Math (per reference):
    I = einsum('bti,oi->bto', spikes, W)         # GEMM (PE, 2 bf16 splits)
    u_t = decay * v_{t-1} + I_t                  # leaky integrate
    s_t = (u_t - 1 > 0)                          # spike (computed on host from u)
    v_t = u_t * (1 - s_t)                        # reset to zero

Sharding: data-parallel over B (32 batches -> 4 per core). Each core runs the
full T=1000 scan for its 4 batches (16 scan lanes of 128 partitions); the u
trajectory is DMA'd out and the host thresholds u > 1 (same bytes as a spike
plane, and bit-exactly equivalent).

v5 changes vs v4 (all HW-microbenchmarked on this pod via For_i loop-delta):
  - Chain sync: the 1000-step recurrence runs as two interleaved 8-lane
    chains (as v4), but each chain instruction's wait on the DVE's own
    semaphore is rewritten post-compile from "previous instruction" to "two
    instructions back" — which IS its RAW producer (chains alternate A,B).
    The producer's semaphore fires only after its SBUF write drains, so
    correctness stays semaphore-guaranteed, while the adjacent-instruction
    handshake comes off the critical path: 214.9us/iter vs 248.7 with tile's
    default 1-back waits (fully stripping the waits measures the same 217
    and loses the guarantee, so the relaxed wait is kept). A dependent
    chain's floor is the engine's ~72ns per-op issue period x2 = ~145ns/step;
    a single 16-lane chain without sems corrupts (~97ns issue < ~129ns SBUF
    write-visibility window), which pins the safe per-step floor at 2 ops.
  - GEMM: one PSUM tile [128, 16 lanes, 128] per chunk (2 ping-pong buffers
    = all 16KB of PSUM). Each lane slot is 512B, so four accumulation
    groups share a 2KB bank — and the PSUM has_written CLEAR issued by
    matmul start= is bank-granular: only the first group per bank (ot==0)
    may pass start=True, or later starts wipe earlier groups' accumulation
    flags (observed as ot 0-2 losing exactly their first matmul's
    contribution). Writes to flag-clear regions overwrite-then-set, which
    covers the other groups' first matmuls.
  - PSUM->SBUF: ONE Activation copy of 16*ch elements per chunk instead of
    16 per-lane copies (the ~370ns SBUF-access bubble per Act op dominated
    at 16 copies x 27 chunks in v4).
  - Chunk schedule: 32 cols ramping x1.25 to 128 (PSUM bank limit), with a
    descending 64/48/32 tail so the final chunk's output DMA (the only
    non-overlapped drain) stays small.
"""

import sys

sys.path.insert(0, "/opt/trn_rl_repo")

import numpy as np
import ml_dtypes

import concourse.bacc as bacc
import concourse.tile as tile
import concourse.mybir as mybir
import concourse.dve_ops as dve_ops
from concourse.dve_ops import DveOp
from concourse.dve_spec import C0, C1, Spec, Src0, Src1, Zero, lower, select
from concourse.dve_uop import DveOpSpec
from concourse.bass_utils import run_bass_kernel_spmd

# ---------------- problem constants (hardcoded from spec) ----------------
B, T, N_IN, N_OUT = 32, 1000, 1024, 512
N_CORES = 8
B_SH = B // N_CORES          # 4 batches per core
DECAY = float(np.exp(-1.0 / 20.0))
# u < nextafter(1.0)  <=>  u <= 1.0 in fp32
THRESH_LT = float(np.nextafter(np.float32(1.0), np.float32(np.inf)))

CH_CAP = 128                 # PSUM: 16 lanes x 128 f32 x 4B = 8KB = 4 banks


def _chunks():
    out, t, ch = [], 0, 32
    # ramp up; leave room for the remainder + descending tail
    while t + ch < T - 176:
        out.append(ch)
        t += ch
        ch = min(int(ch * 1.25), CH_CAP)
    rem = T - t
    tail = [64, 48, 32]       # descending: final (non-overlapped) DMA stays small
    mid = rem - sum(tail)
    while mid > CH_CAP:
        out.append(CH_CAP)
        mid -= CH_CAP
    if mid >= 16:
        out.append(mid)
    elif mid:
        tail[0] += mid
    out += tail
    assert sum(out) == T and all(0 < c <= CH_CAP for c in out), out
    return out


CH_LIST = _chunks()
N_IT = N_IN // 128           # 8 contraction tiles
N_OT = N_OUT // 128          # 4 output-partition tiles
LANES = B_SH * N_OT          # 16 scan lanes per core (free dim)
N_SPLIT = 2                  # bf16 splits of W (hi + mid)


# ---------------- custom DVE op: one LIF step per instruction ----------------
def _lif_ref(in0, in1, c0, c1, c2):
    y = np.where(in0.astype(np.float32) < c1, in0, 0.0).astype(np.float32)
    return (y * np.float32(c0) + in1.astype(np.float32)).astype(np.float32)


_LIF_SPEC = Spec(body=select(Src0 < C1, Src0, Zero) * C0 + Src1, reference=_lif_ref)
_LIF_NAME = "LIF_STEP_ANT"


def _register_lif_op() -> DveOp:
    if _LIF_NAME in dve_ops._SUB_OPCODE_FOR_NAME:
        for op in dve_ops.OPS:
            if op.name == _LIF_NAME:
                return op
    opcode = dve_ops._CUSTOM_DVE_ROW_BASE + len(dve_ops.OPS)
    assert opcode < 0x20
    dve_ops._SUB_OPCODE_FOR_NAME[_LIF_NAME] = opcode
    shas = {}
    for ver in ("v3", "v4"):
        tmp = DveOpSpec(
            name=_LIF_NAME, opcode=opcode, uops=lower(_LIF_SPEC, ver=ver), rd1_en=True
        )
        shas[ver] = tmp.sha(ver)
    op = DveOp(_LIF_NAME, _LIF_SPEC, subdim=False, uops_sha=shas)
    dve_ops.OPS.append(op)
    dve_ops.CUSTOM_DVE_SPECS[_LIF_NAME] = _LIF_SPEC
    return op


# ---------------- device kernel ----------------
def _build_kernel(n_iter=None):
    """Build the per-core program. n_iter wraps the body in a hardware For_i
    loop (used only by the timing probe; production passes None)."""
    import contextlib

    LIF = _register_lif_op()
    nc = bacc.Bacc("TRN2", target_bir_lowering=False, debug=False, num_devices=N_CORES)
    xT = nc.dram_tensor("xT", [B_SH, N_IN, T], mybir.dt.bfloat16, kind="ExternalInput")
    wts = nc.dram_tensor(
        "wts", [N_SPLIT, N_IN, N_OUT], mybir.dt.bfloat16, kind="ExternalInput"
    )
    out = nc.dram_tensor("out", [B_SH, N_OUT, T], mybir.dt.float32, kind="ExternalOutput")
    out_r = out.rearrange("b (ot p) t -> p (b ot) t", p=128)

    with tile.TileContext(nc) as tc:
        with (
            tc.tile_pool(name="wx", bufs=1) as wx_pool,
            tc.tile_pool(name="state", bufs=1) as state_pool,
            tc.tile_pool(name="mm", bufs=2, space="PSUM") as psum_pool,
        ):
            w_sb = wx_pool.tile([128, N_SPLIT, N_IT, N_OUT], mybir.dt.bfloat16, tag="w")
            wts_r = wts.rearrange("s (it p) o -> p s it o", p=128)
            x_sb = [
                wx_pool.tile([128, N_IT, T], mybir.dt.bfloat16, tag=f"x{b}", name=f"x{b}")
                for b in range(B_SH)
            ]
            # Dedicated per-chunk trajectory tiles: the output DMA never
            # blocks the chain via WAR.
            U = [
                state_pool.tile(
                    [128, LANES, ch + 1], mybir.dt.float32, tag=f"U{i}", name=f"U{i}"
                )
                for i, ch in enumerate(CH_LIST)
            ]
            Ibuf = [
                state_pool.tile([128, LANES, CH_CAP], mybir.dt.float32, tag=f"I{k}",
                                name=f"I{k}")
                for k in range(2)
            ]
            zero_col = state_pool.tile([128, LANES], mybir.dt.float32, tag="z")

            loop_cm = tc.For_i(0, n_iter) if n_iter is not None else contextlib.nullcontext()
            with loop_cm:
                nc.vector.memset(zero_col[:], 0.0)
                # DMA order matters (in-order queues): chunk-0 x columns
                # first, then W in use order, then the bulk of x in chunk
                # order so every chunk's columns arrive ahead of its GEMM.
                head = CH_LIST[0]
                for b in range(B_SH):
                    nc.sync.dma_start(
                        x_sb[b][:, :, :head],
                        xT[b].rearrange("(it p) t -> p it t", p=128)[:, :, :head],
                    )
                for s in range(N_SPLIT):
                    for it in range(N_IT):
                        nc.sync.dma_start(w_sb[:, s, it], wts_r[:, s, it])
                tpos = head
                for ch_k in CH_LIST[1:]:
                    for b in range(B_SH):
                        nc.sync.dma_start(
                            x_sb[b][:, :, tpos : tpos + ch_k],
                            xT[b].rearrange("(it p) t -> p it t", p=128)[:, :, tpos : tpos + ch_k],
                        )
                    tpos += ch_k

                t0 = 0
                prev_ch = 0
                for ic, ch in enumerate(CH_LIST):
                    pc = ic % 2
                    # ---- GEMM: one PSUM tile [128, 16, ch]; lane = b*4+ot.
                    # Each lane slot is 128 f32 = 512B (bank-aligned), so
                    # every accumulation group sits inside one PSUM bank.
                    pss = psum_pool.tile(
                        [128, LANES, CH_CAP], mybir.dt.float32, tag="ps", name="ps"
                    )
                    for s in range(N_SPLIT):
                        for it in range(N_IT):
                            for ot in range(N_OT):
                                w_ap = w_sb[:, s, it, ot * 128 : (ot + 1) * 128]
                                for b in range(B_SH):
                                    # start= clears has_written BANK-wide:
                                    # only the first group per bank (ot==0)
                                    # may clear; the others' first writes hit
                                    # flag-clear regions and overwrite-then-
                                    # set, so no start is needed for them.
                                    nc.tensor.matmul(
                                        pss[:, b * N_OT + ot, :ch],
                                        w_ap,
                                        x_sb[b][:, it, t0 : t0 + ch],
                                        start=(s == 0 and it == 0 and ot == 0),
                                        stop=(s == N_SPLIT - 1 and it == N_IT - 1),
                                        skip_group_check=True,
                                    )
                    # ---- PSUM -> SBUF: one Act copy per chunk (16*ch elems)
                    nc.scalar.copy(Ibuf[pc][:, :, :ch], pss[:, :, :ch])

                    # ---- LIF chain: two interleaved 8-lane chains per step.
                    # Self-waits relaxed to the RAW producer post-compile.
                    for j in range(ch):
                        if ic == 0 and j == 0:
                            prev = zero_col
                        elif j == 0:
                            prev = U[ic - 1][:, :, prev_ch]
                        else:
                            prev = U[ic][:, :, j]
                        for lo in (0, 8):
                            nc.vector._custom_dve(
                                LIF,
                                out=U[ic][:, lo : lo + 8, j + 1],
                                in0=prev[:, lo : lo + 8],
                                in1=Ibuf[pc][:, lo : lo + 8, j],
                                s0=DECAY,
                                s1=THRESH_LT,
                            )

                    # ---- stream the u trajectory out; host thresholds u > 1
                    nc.sync.dma_start(
                        out_r[:, :, t0 : t0 + ch],
                        U[ic][:, :, 1 : ch + 1],
                    )
                    t0 += ch
                    prev_ch = ch

    _dedupe_ldweights(nc)
    nrelax = _relax_chain_selfwaits(nc)
    assert nrelax >= 2 * T - len(CH_LIST) - 2, f"relaxed only {nrelax} waits"
    nc.compile()
    return nc


def _relax_chain_selfwaits(nc, back=1):
    """Rewrite each LIF chain instruction's wait on the DVE's own semaphore
    from 'previous instruction' to 'two instructions earlier'. With two
    interleaved 8-lane chains the instruction 2 back IS the RAW producer, so
    correctness stays semaphore-guaranteed (the update fires after the
    producer's write drains) while the adjacent-instruction handshake comes
    off the critical path (~34ns/step on HW). Cross-engine waits (Act->DVE
    chunk-boundary I availability) are untouched; because the engine issues
    in order, a chunk-boundary wait covers every later instruction."""
    n = 0
    for blk in nc.m.functions[0].blocks:
        for inst in blk.instructions:
            if inst.opcode != "ISA" or str(inst.engine) != "EngineType.DVE":
                continue
            si = inst.sync_info
            if not si or not si.on_wait:
                continue
            for w in si.on_wait:
                if w.ant_name and w.ant_name.startswith("DVE") and w.wait_value:
                    w.wait_value = max(0, w.wait_value - back)
                    n += 1
    return n


def _dedupe_ldweights(nc):
    """Remove back-to-back redundant Ldweights.

    The batch-inner GEMM loop issues 4 matmuls per weight tile; bass emits
    an Ldweights per matmul, so 3 of every 4 weight loads re-load the array
    with the bits it already holds. The PE keeps the stationary operand
    until the next Ldweights, so a duplicate load whose weights AP is
    identical to the previous one is a no-op -- drop it, provided it
    carries no semaphore waits/updates and only Matmult instructions sit
    in between."""

    def _key(inst):
        a = inst.ins[0]
        try:
            return (a.memory_location().name, a.offset, str(a.ap))
        except Exception:
            return None

    removed = 0
    for blk in nc.m.functions[0].blocks:
        prev_key = None
        keep = []
        for inst in blk.instructions:
            if inst.opcode == "Ldweights":
                k = _key(inst)
                plain = not inst.sync_info and k is not None
                if plain and k == prev_key:
                    removed += 1
                    continue
                prev_key = k if plain else None
            elif inst.opcode != "Matmult":
                prev_key = None
            keep.append(inst)
        blk.instructions = keep
    return removed


_NC_CACHE = None


def _prep_inputs(input_spikes_seq: np.ndarray, W: np.ndarray):
    W32 = np.ascontiguousarray(np.asarray(W, dtype=np.float32).T)   # [n_in, n_out]
    w_hi = W32.astype(ml_dtypes.bfloat16)
    r1 = W32 - w_hi.astype(np.float32)
    w_mid = r1.astype(ml_dtypes.bfloat16)
    wts = np.ascontiguousarray(np.stack([w_hi, w_mid]))

    x = np.asarray(input_spikes_seq, dtype=np.float32)
    in_maps = []
    for c in range(N_CORES):
        xs = x[c * B_SH : (c + 1) * B_SH]                           # [4, T, n_in]
        xs_T = np.ascontiguousarray(xs.transpose(0, 2, 1)).astype(ml_dtypes.bfloat16)
        in_maps.append({"xT": xs_T, "wts": wts})
    return in_maps


def kernel(input_spikes_seq: np.ndarray, W: np.ndarray) -> np.ndarray:
    global _NC_CACHE
    if _NC_CACHE is None:
        _NC_CACHE = _build_kernel()
    nc = _NC_CACHE

    in_maps = _prep_inputs(input_spikes_seq, W)
    res = run_bass_kernel_spmd(nc, in_maps, core_ids=list(range(N_CORES)))

    # ---- gather/unshard: [core][4, n_out, T] u-values -> spikes (B, T, n_out)
    outs = [r["out"] for r in res.results]
    full_u = np.concatenate(outs, axis=0)                           # [B, n_out, T]
    spikes = (full_u > np.float32(1.0)).astype(np.float32)
    return np.ascontiguousarray(spikes.transpose(0, 2, 1))
